# revision 40
# baseline (speedup 1.0000x reference)
"""Trainium2 Bass kernel for nn_Attention_62620623176132.

Multi-head causal attention with RoPE (LLaMA-style), B=2, S=2048, D=2048,
H=16 heads of HD=128, fp32 reference.

Sharding (hardcoded): 8 cores = 2-way data parallel over batch x 4-way
tensor parallel over heads (4 heads per core). Each core computes its 4
heads' Q/K/V projections, attention, and a partial output projection
(rows of wo for its heads); the host sums the 4 fp16 partials per batch
in fp32.

Device algorithm (per core; matmuls in fp16 with fp32 PSUM accumulation):
  - x^T kept SBUF-resident; Q^T/K^T computed per head in [HD, S] layout,
    V in [S, dv] layout, so no transposes are ever needed.
  - RoPE via host-side even/odd column permutation of wq/wk: rotation
    pairs land in partition halves; 3 DVE tensor ops + 2 swap copies.
    In the startup loop RoPE runs per 512-token chunk, interleaved with
    the V projections, so the DVE chains hide under PE work.
  - Scores computed transposed, sT[kt, qt] = kT . qT, so exp(sT) feeds
    the PV matmul directly as the moving operand. exp is shifted by -4
    (softmax is shift-invariant) to keep fp16 outputs far from overflow.
  - Softmax denominators: exp tiles are summed on the Vector engine and
    one all-ones stationary matmul per query chunk broadcasts the column
    sums to all partitions; the normalization multiply is fused into the
    PSUM->SBUF copy of the attention output.
  - Causality: score tiles above the diagonal are skipped; band tiles
    are restricted to their unmasked columns and the diagonal square of
    exp values is multiplied by a 0/1 triangle on the Vector engine, so
    masking costs no PE work.
  - Projections for head h+1 are emitted inside head h's attention so
    the serial RoPE chain never stalls the PE; the output projection is
    streamed inside the last head's attention the same way.
  - Output tiles are staged per 128-token tile into a [P, 2048] tile
    and written with a single 4KB-row DMA (4x fewer ~0.6us DMA issues).
  - Startup DMAs are split per 4-kc group and spread across the Sync and
    Scalar hardware-DGE queues in consumption order, so the first K-proj
    matmuls start as soon as the first x^T group lands. (GpSimd DMA is
    the software-DGE path: its transfers start late and stall the Pool
    queue -- never put startup or output DMAs there.)

Fallback paths keyed off the runtime mask: all-zero mask -> non-causal
kernel; any other mask -> multiplicative exp(mask/sqrt(HD)) tiles
streamed from DRAM (correct for arbitrary masks, slower).
"""

import math

import numpy as np
import concourse.tile as tile
import concourse.mybir as mybir
from concourse import bacc
from concourse.bass import ts
from concourse.bass_utils import run_bass_kernel_spmd

B, S, D, H, HD = 2, 2048, 2048, 16, 128
P = 128
NCORES = 8
TP = 4                # head-parallel groups
HPC = H // TP         # heads per core = 4
DVC = HPC * HD        # 512 v-dims per core
KC = D // P           # 16 contraction chunks
NT = S // P           # 16 token tiles of 128
NQ = S // 512         # 4 query chunks of 512
F16 = mybir.dt.float16
F32 = mybir.dt.float32
NPF16 = np.float16
SCALE = 1.0 / math.sqrt(HD)
EXP = mybir.ActivationFunctionType.Exp

_cache: dict = {}


def _build(mask_mode: str):
    """Build + compile the SPMD program. mask_mode: 'causal'|'none'|'general'."""
    nc = bacc.Bacc("TRN2", target_bir_lowering=False, debug=False,
                   num_devices=NCORES)

    def din(name, shape, dt=F16):
        return nc.dram_tensor(name, shape, dt, kind="ExternalInput").ap()

    xT_d = din("xT", [P, NQ, KC, 512])
    wq_d = din("wq", [P, HPC, KC, HD])
    wk_d = din("wk", [P, HPC, KC, HD])
    wv_d = din("wv", [P, KC, DVC])
    wo_d = din("wo", [P, HPC, D])
    c2_d = din("c2", [P, S])
    s2n_d = din("s2n", [P, S])
    ones_d = din("ones", [P, P])
    if mask_mode == "causal":
        tri_d = din("tri", [P, P])
    elif mask_mode == "general":
        msk_d = din("expm", [P, NT, S])
    out_d = nc.dram_tensor("out", [P, NT, D], mybir.dt.float16,
                           kind="ExternalOutput").ap()

    with tile.TileContext(nc) as tc:
        with tc.tile_pool(name="static", bufs=1) as st, \
             tc.tile_pool(name="w1", bufs=1) as w1, \
             tc.tile_pool(name="w2", bufs=2) as w2, \
             tc.tile_pool(name="et", bufs=6) as etp, \
             tc.tile_pool(name="ac", bufs=3) as accp, \
             tc.tile_pool(name="fo", bufs=3) as fop, \
             tc.tile_pool(name="ri", bufs=2) as rip, \
             tc.tile_pool(name="pj", bufs=2, space="PSUM") as pjp:

            # ---- static tensors -------------------------------------------
            xT = st.tile([P, NQ, KC, 512], F16, tag="xT")
            wv_sb = st.tile([P, KC, DVC], F16, tag="wv")
            wo_sb = st.tile([P, HPC, D], F16, tag="wo")
            c2 = st.tile([P, S], F16, tag="c2")
            s2n = st.tile([P, S], F16, tag="s2n")
            ones_sb = st.tile([P, P], F16, tag="ones")
            V_sb = st.tile([P, NT, DVC], F16, tag="V")
            OT_sb = st.tile([P, HPC, S], F16, tag="OT")
            if mask_mode == "causal":
                tri_sb = st.tile([P, P], F16, tag="tri")
            bias4 = st.tile([P, 1], F32, tag="b4")
            nc.vector.memset(bias4[:], -4.0)

            # Startup DMAs: head-0 wk/wq and xT chunk 0 split per 4-kc group
            # so the first projection matmuls start after ~640KB; issues are
            # spread across four engine queues (each costs ~0.6us of issue
            # time on its queue).
            wq_h = w1.tile([P, KC, HD], F16, tag="wqh")
            wk_h = w1.tile([P, KC, HD], F16, tag="wkh")
            # Scalar gets ONLY the 4 small wq chunks: a ring-full DMA wait on
            # the Scalar queue would block the projection PSUM->SBUF copies
            # behind it and starve the PE of PSUM banks. Everything else goes
            # on Sync in strict arrival-priority order (ring-full waits there
            # only delay later, less urgent transfers).
            nc.sync.dma_start(wk_h[:, ts(0, 4), :], wk_d[:, 0, ts(0, 4), :])
            nc.sync.dma_start(xT[:, 0, 0:2, :], xT_d[:, 0, 0:2, :])
            nc.scalar.dma_start(wq_h[:, ts(0, 4), :], wq_d[:, 0, ts(0, 4), :])
            nc.sync.dma_start(xT[:, 0, 2:4, :], xT_d[:, 0, 2:4, :])
            for g in range(1, 4):
                nc.sync.dma_start(wk_h[:, ts(g, 4), :], wk_d[:, 0, ts(g, 4), :])
                nc.sync.dma_start(xT[:, 0, 4 * g:4 * g + 2, :],
                                  xT_d[:, 0, 4 * g:4 * g + 2, :])
                nc.sync.dma_start(xT[:, 0, 4 * g + 2:4 * g + 4, :],
                                  xT_d[:, 0, 4 * g + 2:4 * g + 4, :])
                nc.scalar.dma_start(wq_h[:, ts(g, 4), :], wq_d[:, 0, ts(g, 4), :])
            for g in range(4):
                nc.sync.dma_start(wv_sb[:, ts(g, 4), :], wv_d[:, ts(g, 4), :])
            nc.sync.dma_start(c2[:], c2_d)
            nc.sync.dma_start(s2n[:], s2n_d)
            nc.sync.dma_start(xT[:, 1, :, :], xT_d[:, 1, :, :])
            nc.sync.dma_start(xT[:, 2, :, :], xT_d[:, 2, :, :])
            nc.sync.dma_start(xT[:, 3, :, :], xT_d[:, 3, :, :])
            nc.sync.dma_start(ones_sb[:], ones_d)
            if mask_mode == "causal":
                nc.sync.dma_start(tri_sb[:], tri_d)
            nc.sync.dma_start(wo_sb[:], wo_d)

            # PE warm-up: garbage matmuls on a zeroed scratch tile (output
            # never read) fill the ~4us DMA-start latency window so the PE
            # p-state is fully ramped when the first real operands land.
            # (5 matmuls / DVE memset measured best: longer warm-up chains
            # delay the first real matmuls in the in-order PE queue by more
            # than the p-state ramp they save)
            scr = st.tile([P, 512], F16, tag="scr")
            nc.vector.memset(scr[:], 0.0)
            warm_ps = pjp.tile([P, 512], F32, tag="pj")
            for _ in range(5):
                nc.tensor.matmul(warm_ps[:], scr[:, 0:P], scr[:],
                                 start=True, stop=True)

            def rope_chunk(raw, rot, t0, ntc):
                """RoPE on token chunks [t0*512, (t0+ntc)*512) of raw -> rot."""
                lo, n = 512 * t0, 512 * ntc
                swp = w1.tile([P, S], F16, tag="swap")
                nc.vector.tensor_copy(swp[0:64, lo:lo + n], raw[64:128, lo:lo + n])
                nc.vector.tensor_copy(swp[64:128, lo:lo + n], raw[0:64, lo:lo + n])
                nc.vector.tensor_mul(rot[:, lo:lo + n], raw[:, lo:lo + n],
                                     c2[:, lo:lo + n])
                nc.vector.tensor_mul(swp[:, lo:lo + n], swp[:, lo:lo + n],
                                     s2n[:, lo:lo + n])
                nc.vector.tensor_add(rot[:, lo:lo + n], rot[:, lo:lo + n],
                                     swp[:, lo:lo + n])

            def proj_half(w_h, raw_tag, rot_tag):
                """One projection (Q or K) + RoPE -> rotated [HD, S] tile."""
                raw = w1.tile([P, S], F16, tag=raw_tag)
                for t in range(NQ):
                    ps = pjp.tile([P, 512], F32, tag="pj")
                    for kc in range(KC):
                        nc.tensor.matmul(ps[:], w_h[:, kc, :],
                                         xT[:, t, kc, :],
                                         start=(kc == 0), stop=(kc == KC - 1))
                    nc.scalar.copy(raw[:, ts(t, 512)], ps[:])
                rot = w2.tile([P, S], F16, tag=rot_tag)
                rope_chunk(raw, rot, 0, NQ)
                return rot

            def load_w(h):
                wq_h = w1.tile([P, KC, HD], F16, tag="wqh")
                nc.sync.dma_start(wq_h[:], wq_d[:, h])
                wk_h = w1.tile([P, KC, HD], F16, tag="wkh")
                nc.sync.dma_start(wk_h[:], wk_d[:, h])
                return wq_h, wk_h

            # ---- head-0 projections + V, interleaved ----------------------
            # Per token chunk: K then Q (RoPE chunks run on DVE right after),
            # then the V projections keep the PE busy while DVE rotates and
            # the next xT chunk streams in.
            qraw0 = w1.tile([P, S], F16, tag="qraw")
            kraw0 = w1.tile([P, S], F16, tag="kraw")
            qrot0 = w2.tile([P, S], F16, tag="qrot")
            krot0 = w2.tile([P, S], F16, tag="krot")
            for t in range(NQ):
                for w_h, raw in ((wk_h, kraw0), (wq_h, qraw0)):
                    ps = pjp.tile([P, 512], F32, tag="pj")
                    for kc in range(KC):
                        nc.tensor.matmul(ps[:], w_h[:, kc, :], xT[:, t, kc, :],
                                         start=(kc == 0), stop=(kc == KC - 1))
                    nc.scalar.copy(raw[:, ts(t, 512)], ps[:])
                rope_chunk(kraw0, krot0, t, 1)
                rope_chunk(qraw0, qrot0, t, 1)
                for ti in range(4 * t, 4 * t + 4):
                    ps = pjp.tile([P, 512], F32, tag="pj")
                    for kc in range(KC):
                        nc.tensor.matmul(ps[:], xT[:, t, kc, ts(ti % 4, P)],
                                         wv_sb[:, kc, :],
                                         start=(kc == 0), stop=(kc == KC - 1))
                    nc.scalar.copy(V_sb[:, ti, :], ps[:])
            rots = {0: (qrot0, krot0)}

            flip = [0]

            def emit_f(qi, pool, engines=("vector", "scalar"), split_dma=False):
                """Output projection for token tile qi -> one [P, 2048] DMA.

                engines: PSUM->SBUF copy engines to rotate through.
                (GpSimd cannot access PSUM -- walrus rejects it.)
                """
                f_sb = fop.tile([P, D], F16, tag="fsb")
                for nn in range(D // 512):
                    ps = pool.tile([P, 512], F32, tag="pj")
                    for hh in range(HPC):
                        nc.tensor.matmul(ps[:], OT_sb[:, hh, ts(qi, P)],
                                         wo_sb[:, hh, ts(nn, 512)],
                                         start=(hh == 0), stop=(hh == HPC - 1))
                    eng = engines[flip[0] % len(engines)]
                    if eng == "vector":
                        nc.vector.tensor_copy(f_sb[:, ts(nn, 512)], ps[:])
                    elif eng == "gpsimd":
                        nc.gpsimd.tensor_copy(f_sb[:, ts(nn, 512)], ps[:])
                    else:
                        nc.scalar.copy(f_sb[:, ts(nn, 512)], ps[:])
                    flip[0] += 1
                    # final tiles: drain each half through its own
                    # hardware-DGE queue as soon as its copies are done
                    if split_dma and nn == 1:
                        nc.sync.dma_start(out_d[:, qi, 0:1024], f_sb[:, 0:1024])
                    if split_dma and nn == 3:
                        nc.scalar.dma_start(out_d[:, qi, 1024:2048],
                                            f_sb[:, 1024:2048])
                if not split_dma:
                    nc.sync.dma_start(out_d[:, qi, :], f_sb[:])

            # ---- attention, with next head's projections interleaved ------
            with tc.tile_pool(name="sp", bufs=3, space="PSUM") as stp, \
                 tc.tile_pool(name="op", bufs=2, space="PSUM") as opp, \
                 tc.tile_pool(name="rp", bufs=1, space="PSUM") as rpp:
                pending_fin = [None]
                carry = [[]]
                for h in range(HPC):
                    qrot, krot = rots.pop(h)
                    last = h == HPC - 1
                    # independent PE work units used to plug exp-latency
                    # bubbles in the in-order PE queue: next head's projection
                    # chunks (heads 0..2, during qc==0) or the previous query
                    # chunk's output-projection tiles (last head).
                    if not last:
                        wq_n, wk_n = load_w(h + 1)
                        qraw_n = w1.tile([P, S], F16, tag="qraw")
                        kraw_n = w1.tile([P, S], F16, tag="kraw")
                        qrot_n = w2.tile([P, S], F16, tag="qrot")
                        krot_n = w2.tile([P, S], F16, tag="krot")
                        rots[h + 1] = (qrot_n, krot_n)

                        def proj_unit(w_h, raw, rot, t):
                            def emit():
                                ps = pjp.tile([P, 512], F32, tag="pj")
                                for kc in range(KC):
                                    nc.tensor.matmul(ps[:], w_h[:, kc, :],
                                                     xT[:, t, kc, :],
                                                     start=(kc == 0),
                                                     stop=(kc == KC - 1))
                                nc.scalar.copy(raw[:, ts(t, 512)], ps[:])
                                rope_chunk(raw, rot, t, 1)
                            return emit

                        units = [proj_unit(wk_n, kraw_n, krot_n, t)
                                 for t in range(NQ)]
                        units += [proj_unit(wq_n, qraw_n, qrot_n, t)
                                  for t in range(NQ)]
                        if h == HPC - 2:
                            # hold back two of the last head's projection
                            # chunks: they are its only filler work for the
                            # serial band chain of its first query chunk
                            carry[0] = units[6:]
                            units = units[:6]
                    else:
                        units = carry[0]
                    for qc in range(NQ):
                        o_ps = opp.tile([P, 512], F32, tag="o")
                        r_ps = None
                        if mask_mode != "causal":
                            r_ps = rpp.tile([P, 512], F32, tag="r")
                        nkt = 4 * (qc + 1) if mask_mode == "causal" else NT
                        nfull = 4 * qc if mask_mode == "causal" else 0
                        if last and mask_mode == "causal" and qc > 0:
                            units = [(lambda qi: lambda: emit_f(qi, pjp))(qi)
                                     for qi in range(4 * (qc - 1), 4 * qc)]
                        # full-width tiles accumulate on the DVE into `acc`;
                        # one ones-matmul on the sum replaces one per tile.
                        state = {"acc": None, "first_e": None}
                        pend = {}

                        def emit_score(kt):
                            band = mask_mode == "causal" and kt >= nfull
                            off = 128 * (kt - nfull) if band else 0
                            s_ps = stp.tile([P, 512], F32, tag="s")
                            nc.tensor.matmul(
                                s_ps[:, off:], krot[:, ts(kt, P)],
                                qrot[:, 512 * qc + off: 512 * (qc + 1)],
                                start=True, stop=True)
                            eT = etp.tile([P, 512], F16, tag="e")
                            # bias -4 (softmax is shift-invariant; the ones-
                            # matmul denominator absorbs it) keeps exp outputs
                            # well inside fp16 range even for hot scores
                            nc.scalar.activation(eT[:, off:], s_ps[:, off:], EXP,
                                                 scale=SCALE, bias=bias4[:])
                            if band:
                                # zero the above-diagonal half of the diagonal
                                # 128x128 block (cheap on DVE; GpSimd takes
                                # ~5x longer per op and SWDGE-stalls its queue)
                                nc.vector.tensor_mul(eT[:, off:off + 128],
                                                     eT[:, off:off + 128],
                                                     tri_sb[:])
                            if mask_mode == "general":
                                em = etp.tile([P, 512], F16, tag="em")
                                nc.sync.dma_start(em[:], msk_d[:, kt, ts(qc, 512)])
                                nc.gpsimd.tensor_mul(eT[:], eT[:], em[:])
                            acc, first_e = state["acc"], state["first_e"]
                            if mask_mode != "causal":
                                pass
                            elif not band:
                                if first_e is not None:
                                    acc = accp.tile([P, 512], F16, tag="acc")
                                    nc.vector.tensor_add(acc[:], first_e[:], eT[:])
                                    state["acc"], state["first_e"] = acc, None
                                elif acc is not None:
                                    nc.vector.tensor_add(acc[:], acc[:], eT[:])
                                else:
                                    state["first_e"] = eT
                            elif acc is None and first_e is None:
                                acc = accp.tile([P, 512], F16, tag="acc")
                                nc.vector.tensor_copy(acc[:], eT[:])
                                state["acc"] = acc
                            elif first_e is not None:
                                acc = accp.tile([P, 512], F16, tag="acc")
                                nc.vector.tensor_add(acc[:], first_e[:], eT[:])
                                state["acc"], state["first_e"] = acc, None
                            else:
                                nc.vector.tensor_add(acc[:, off:], acc[:, off:],
                                                     eT[:, off:])
                            pend[kt] = (eT, off)

                        # software-pipeline by one tile: the PE sees
                        # [s0, s1, PV0, s2, PV1, ...] so PV(kt) never waits on
                        # exp(kt) -- the next score matmul runs in the gap.
                        # (Depth 2 regresses badly: a third in-flight score
                        # tile exhausts the sp ring and the allocation wait
                        # serializes the whole PE queue.)
                        emit_score(0)
                        for kt in range(nkt):
                            if kt + 1 < nkt:
                                emit_score(kt + 1)
                            # the previous chunk's denominator chain (ones
                            # matmul -> reciprocal -> OT multiply) runs here,
                            # inside this chunk's score stream, so the PE
                            # never serializes behind the DVE acc chain at a
                            # chunk boundary
                            if kt == 1 and pending_fin[0] is not None:
                                pending_fin[0]()
                                pending_fin[0] = None
                            eT, off = pend.pop(kt)
                            nc.tensor.matmul(o_ps[:, off:],
                                             V_sb[:, kt, ts(h, HD)], eT[:, off:],
                                             start=(kt == 0), stop=(kt == nkt - 1))
                            if mask_mode != "causal":
                                nc.tensor.matmul(r_ps[:], ones_sb[:], eT[:],
                                                 start=(kt == 0),
                                                 stop=(kt == nkt - 1))
                            # last-head emit units read OT written at the END
                            # of the previous chunk (after its DVE rinv+mul
                            # chain): schedule them from kt=4 so the first one
                            # never stalls the PE on that chain
                            # qc==0 is a serial band chain: fire filler at
                            # kt 0/2 so the unit absorbs the exp+tri latency
                            # of the remaining band tiles instead of running
                            # after the bubbles have already accrued
                            if qc == 0:
                                ready = kt in (0, 2)
                            elif last:
                                ready = kt >= 5 and (kt - 5) % 3 == 0
                            else:
                                ready = kt % 3 == 2
                            if units and ready:
                                units.pop(0)()
                        # the last head's output tiles must flush before the
                        # next query chunk overwrites fq context; projection
                        # units may keep spreading across later query chunks
                        if last:
                            for u in units:
                                u()
                            units = []
                        if mask_mode == "causal":
                            def finalize(h=h, qc=qc, o_ps=o_ps,
                                         acc=state["acc"]):
                                r_ps = rpp.tile([P, 512], F32, tag="r")
                                nc.tensor.matmul(r_ps[:], ones_sb[:], acc[:],
                                                 start=True, stop=True)
                                rinv = rip.tile([P, 512], F32, tag="rinv")
                                nc.vector.reciprocal_approx_fast(out=rinv[:],
                                                                 in_=r_ps[:])
                                nc.vector.tensor_mul(OT_sb[:, h, ts(qc, 512)],
                                                     o_ps[:], rinv[:])
                            if last and qc == NQ - 1:
                                finalize()
                            else:
                                pending_fin[0] = finalize
                        else:
                            rinv = rip.tile([P, 512], F32, tag="rinv")
                            nc.vector.reciprocal_approx_fast(out=rinv[:],
                                                             in_=r_ps[:])
                            nc.vector.tensor_mul(OT_sb[:, h, ts(qc, 512)],
                                                 o_ps[:], rinv[:])
                    # any projection units not consumed by the kt loops
                    for u in units:
                        u()
                    if last and mask_mode == "causal":
                        for qi in range(4 * (NQ - 1), NT):
                            emit_f(qi, pjp, split_dma=True)

            # ---- output projection for non-causal modes (causal streams it
            # inside the last head's attention) --------------------------------
            if mask_mode != "causal":
                with tc.tile_pool(name="fp", bufs=6, space="PSUM") as fpp:
                    for qi in range(NT):
                        emit_f(qi, fpp, engines=("scalar", "vector"))

    nc.compile()
    return nc


def _get_program(mask_mode: str):
    if mask_mode not in _cache:
        _cache[mask_mode] = _build(mask_mode)
    return _cache[mask_mode]


def _detect_mask_mode(mask: np.ndarray) -> str:
    m = mask.reshape(S, S)
    iu = np.triu_indices(S, 1)
    upper = m[iu]
    lower_ok = np.max(np.abs(np.tril(m))) == 0.0
    if lower_ok and upper.size and np.all(upper <= -1e8):
        return "causal"
    if np.max(np.abs(m)) == 0.0:
        return "none"
    return "general"


def _prep_inputs(x, wq, wk, wv, wo, freqs_cos, freqs_sin, mask, mask_mode):
    """Build the 8 per-core input maps (host-side sharding + layout)."""
    # within-head even/odd permutation so RoPE pairs land in partition halves
    perm = np.concatenate([np.arange(0, HD, 2), np.arange(1, HD, 2)])

    cosT = freqs_cos.T.astype(np.float32)          # [64, S]
    sinT = freqs_sin.T.astype(np.float32)
    c2 = np.concatenate([cosT, cosT], 0).astype(NPF16)     # [128, S]
    s2n = np.concatenate([-sinT, sinT], 0).astype(NPF16)
    ones = np.ones((P, P), NPF16)

    common = {"c2": c2, "s2n": s2n, "ones": ones}
    if mask_mode == "causal":
        pp, ff = np.meshgrid(np.arange(P), np.arange(P), indexing="ij")
        common["tri"] = (pp <= ff).astype(NPF16)
    elif mask_mode == "general":
        m = mask.reshape(S, S).astype(np.float32)
        # eT[kt_tok, qt_tok] is multiplied by exp(SCALE * mask[qt_tok, kt_tok])
        expm = np.exp(SCALE * m.T).astype(NPF16)            # [k_tok, q_tok]
        common["expm"] = np.ascontiguousarray(
            expm.reshape(NT, P, S).transpose(1, 0, 2))

    xT_by_b = []
    for b in range(B):
        xT = np.ascontiguousarray(
            x[b].T.reshape(KC, P, NQ, 512).transpose(1, 2, 0, 3)).astype(NPF16)
        xT_by_b.append(xT)

    in_maps = []
    for c in range(NCORES):
        b, g = divmod(c, TP)
        heads = range(g * HPC, (g + 1) * HPC)
        cols_qk = np.concatenate([h * HD + perm for h in heads])
        cols_v = np.concatenate([np.arange(h * HD, (h + 1) * HD) for h in heads])

        wq_c = wq[:, cols_qk].reshape(KC, P, HPC, HD).transpose(1, 2, 0, 3)
        wk_c = wk[:, cols_qk].reshape(KC, P, HPC, HD).transpose(1, 2, 0, 3)
        wv_c = wv[:, cols_v].reshape(KC, P, DVC).transpose(1, 0, 2)
        wo_c = wo[cols_v, :].reshape(HPC, P, D).transpose(1, 0, 2)

        im = dict(common)
        im["xT"] = xT_by_b[b]
        im["wq"] = np.ascontiguousarray(wq_c).astype(NPF16)
        im["wk"] = np.ascontiguousarray(wk_c).astype(NPF16)
        im["wv"] = np.ascontiguousarray(wv_c).astype(NPF16)
        im["wo"] = np.ascontiguousarray(wo_c).astype(NPF16)
        in_maps.append(im)
    return in_maps


def run(inputs: dict, **spmd_kwargs):
    """Run on hardware; returns (output [B,S,D] fp32, BassKernelResults)."""
    x = np.asarray(inputs["x"], np.float32)
    wq = np.asarray(inputs["wq"], np.float32)
    wk = np.asarray(inputs["wk"], np.float32)
    wv = np.asarray(inputs["wv"], np.float32)
    wo = np.asarray(inputs["wo"], np.float32)
    fc = np.asarray(inputs["freqs_cos"], np.float32)
    fs = np.asarray(inputs["freqs_sin"], np.float32)
    mask = np.asarray(inputs["mask"], np.float32)

    mask_mode = _detect_mask_mode(mask)
    nc = _get_program(mask_mode)
    in_maps = _prep_inputs(x, wq, wk, wv, wo, fc, fs, mask, mask_mode)
    res = run_bass_kernel_spmd(nc, in_maps, core_ids=list(range(NCORES)),
                               **spmd_kwargs)

    out = np.zeros((B, S, D), np.float32)
    for c in range(NCORES):
        b = c // TP
        part = res.results[c]["out"].astype(np.float32)   # [P, NT, D]
        out[b] += part.transpose(1, 0, 2).reshape(S, D)
    return out, res


def kernel(**inputs) -> np.ndarray:
    out, _ = run(inputs)
    return out


# revision 41
# speedup vs baseline: 1.0019x; 1.0019x over previous
"""Trainium2 Bass kernel for nn_Attention_62620623176132.

Multi-head causal attention with RoPE (LLaMA-style), B=2, S=2048, D=2048,
H=16 heads of HD=128, fp32 reference.

Sharding (hardcoded): 8 cores = 2-way data parallel over batch x 4-way
tensor parallel over heads (4 heads per core). Each core computes its 4
heads' Q/K/V projections, attention, and a partial output projection
(rows of wo for its heads); the host sums the 4 fp16 partials per batch
in fp32.

Device algorithm (per core; matmuls in fp16 with fp32 PSUM accumulation):
  - x^T kept SBUF-resident; Q^T/K^T computed per head in [HD, S] layout,
    V in [S, dv] layout, so no transposes are ever needed.
  - RoPE via host-side even/odd column permutation of wq/wk: rotation
    pairs land in partition halves; 3 DVE tensor ops + 2 swap copies.
    In the startup loop RoPE runs per 512-token chunk, interleaved with
    the V projections, so the DVE chains hide under PE work.
  - Scores computed transposed, sT[kt, qt] = kT . qT, so exp(sT) feeds
    the PV matmul directly as the moving operand. exp is shifted by -4
    (softmax is shift-invariant) to keep fp16 outputs far from overflow.
  - Softmax denominators: exp tiles are summed on the Vector engine and
    one all-ones stationary matmul per query chunk broadcasts the column
    sums to all partitions; the normalization multiply is fused into the
    PSUM->SBUF copy of the attention output.
  - Causality: score tiles above the diagonal are skipped; band tiles
    are restricted to their unmasked columns and the diagonal square of
    exp values is multiplied by a 0/1 triangle on the Vector engine, so
    masking costs no PE work.
  - Projections for head h+1 are emitted inside head h's attention so
    the serial RoPE chain never stalls the PE; the output projection is
    streamed inside the last head's attention the same way.
  - Output tiles are staged per 128-token tile into a [P, 2048] tile
    and written with a single 4KB-row DMA (4x fewer ~0.6us DMA issues).
  - Startup DMAs are split per 4-kc group and spread across the Sync and
    Scalar hardware-DGE queues in consumption order, so the first K-proj
    matmuls start as soon as the first x^T group lands. (GpSimd DMA is
    the software-DGE path: its transfers start late and stall the Pool
    queue -- never put startup or output DMAs there.)

Fallback paths keyed off the runtime mask: all-zero mask -> non-causal
kernel; any other mask -> multiplicative exp(mask/sqrt(HD)) tiles
streamed from DRAM (correct for arbitrary masks, slower).
"""

import math

import numpy as np
import concourse.tile as tile
import concourse.mybir as mybir
from concourse import bacc
from concourse.bass import ts
from concourse.bass_utils import run_bass_kernel_spmd

B, S, D, H, HD = 2, 2048, 2048, 16, 128
P = 128
NCORES = 8
TP = 4                # head-parallel groups
HPC = H // TP         # heads per core = 4
DVC = HPC * HD        # 512 v-dims per core
KC = D // P           # 16 contraction chunks
NT = S // P           # 16 token tiles of 128
NQ = S // 512         # 4 query chunks of 512
F16 = mybir.dt.float16
F32 = mybir.dt.float32
NPF16 = np.float16
SCALE = 1.0 / math.sqrt(HD)
EXP = mybir.ActivationFunctionType.Exp

_cache: dict = {}


def _build(mask_mode: str):
    """Build + compile the SPMD program. mask_mode: 'causal'|'none'|'general'."""
    nc = bacc.Bacc("TRN2", target_bir_lowering=False, debug=False,
                   num_devices=NCORES)

    def din(name, shape, dt=F16):
        return nc.dram_tensor(name, shape, dt, kind="ExternalInput").ap()

    xT_d = din("xT", [P, NQ, KC, 512])
    wq_d = din("wq", [P, HPC, KC, HD])
    wk_d = din("wk", [P, HPC, KC, HD])
    wv_d = din("wv", [P, KC, DVC])
    wo_d = din("wo", [P, HPC, D])
    c2_d = din("c2", [P, S])
    s2n_d = din("s2n", [P, S])
    ones_d = din("ones", [P, P])
    if mask_mode == "causal":
        tri_d = din("tri", [P, P])
    elif mask_mode == "general":
        msk_d = din("expm", [P, NT, S])
    out_d = nc.dram_tensor("out", [P, NT, D], mybir.dt.float16,
                           kind="ExternalOutput").ap()

    with tile.TileContext(nc) as tc:
        with tc.tile_pool(name="static", bufs=1) as st, \
             tc.tile_pool(name="w1", bufs=1) as w1, \
             tc.tile_pool(name="w2", bufs=2) as w2, \
             tc.tile_pool(name="et", bufs=6) as etp, \
             tc.tile_pool(name="ac", bufs=3) as accp, \
             tc.tile_pool(name="fo", bufs=3) as fop, \
             tc.tile_pool(name="ri", bufs=2) as rip, \
             tc.tile_pool(name="pj", bufs=2, space="PSUM") as pjp:

            # ---- static tensors -------------------------------------------
            xT = st.tile([P, NQ, KC, 512], F16, tag="xT")
            wv_sb = st.tile([P, KC, DVC], F16, tag="wv")
            wo_sb = st.tile([P, HPC, D], F16, tag="wo")
            c2 = st.tile([P, S], F16, tag="c2")
            s2n = st.tile([P, S], F16, tag="s2n")
            ones_sb = st.tile([P, P], F16, tag="ones")
            V_sb = st.tile([P, NT, DVC], F16, tag="V")
            OT_sb = st.tile([P, HPC, S], F16, tag="OT")
            if mask_mode == "causal":
                tri_sb = st.tile([P, P], F16, tag="tri")
            bias4 = st.tile([P, 1], F32, tag="b4")
            nc.vector.memset(bias4[:], -4.0)

            # Startup DMAs: head-0 wk/wq and xT chunk 0 split per 4-kc group
            # so the first projection matmuls start after ~640KB; issues are
            # spread across four engine queues (each costs ~0.6us of issue
            # time on its queue).
            wq_h = w1.tile([P, KC, HD], F16, tag="wqh")
            wk_h = w1.tile([P, KC, HD], F16, tag="wkh")
            # Scalar gets ONLY the 4 small wq chunks: a ring-full DMA wait on
            # the Scalar queue would block the projection PSUM->SBUF copies
            # behind it and starve the PE of PSUM banks. Everything else goes
            # on Sync in strict arrival-priority order (ring-full waits there
            # only delay later, less urgent transfers).
            nc.sync.dma_start(wk_h[:, ts(0, 4), :], wk_d[:, 0, ts(0, 4), :])
            nc.sync.dma_start(xT[:, 0, 0:2, :], xT_d[:, 0, 0:2, :])
            nc.scalar.dma_start(wq_h[:, ts(0, 4), :], wq_d[:, 0, ts(0, 4), :])
            nc.sync.dma_start(xT[:, 0, 2:4, :], xT_d[:, 0, 2:4, :])
            for g in range(1, 4):
                nc.sync.dma_start(wk_h[:, ts(g, 4), :], wk_d[:, 0, ts(g, 4), :])
                nc.sync.dma_start(xT[:, 0, 4 * g:4 * g + 2, :],
                                  xT_d[:, 0, 4 * g:4 * g + 2, :])
                nc.sync.dma_start(xT[:, 0, 4 * g + 2:4 * g + 4, :],
                                  xT_d[:, 0, 4 * g + 2:4 * g + 4, :])
                nc.scalar.dma_start(wq_h[:, ts(g, 4), :], wq_d[:, 0, ts(g, 4), :])
            for g in range(4):
                nc.sync.dma_start(wv_sb[:, ts(g, 4), :], wv_d[:, ts(g, 4), :])
            nc.sync.dma_start(c2[:], c2_d)
            nc.sync.dma_start(s2n[:], s2n_d)
            nc.sync.dma_start(xT[:, 1, :, :], xT_d[:, 1, :, :])
            nc.sync.dma_start(xT[:, 2, :, :], xT_d[:, 2, :, :])
            nc.sync.dma_start(xT[:, 3, :, :], xT_d[:, 3, :, :])
            nc.sync.dma_start(ones_sb[:], ones_d)
            if mask_mode == "causal":
                nc.sync.dma_start(tri_sb[:], tri_d)
            nc.sync.dma_start(wo_sb[:], wo_d)

            # PE warm-up: garbage matmuls on a zeroed scratch tile (output
            # never read) fill the ~4us DMA-start latency window so the PE
            # p-state is fully ramped when the first real operands land.
            # (5 matmuls / DVE memset measured best: longer warm-up chains
            # delay the first real matmuls in the in-order PE queue by more
            # than the p-state ramp they save)
            scr = st.tile([P, 512], F16, tag="scr")
            nc.vector.memset(scr[:], 0.0)
            warm_ps = pjp.tile([P, 512], F32, tag="pj")
            for _ in range(5):
                nc.tensor.matmul(warm_ps[:], scr[:, 0:P], scr[:],
                                 start=True, stop=True)

            def rope_chunk(raw, rot, t0, ntc):
                """RoPE on token chunks [t0*512, (t0+ntc)*512) of raw -> rot."""
                lo, n = 512 * t0, 512 * ntc
                swp = w1.tile([P, S], F16, tag="swap")
                nc.vector.tensor_copy(swp[0:64, lo:lo + n], raw[64:128, lo:lo + n])
                nc.vector.tensor_copy(swp[64:128, lo:lo + n], raw[0:64, lo:lo + n])
                nc.vector.tensor_mul(rot[:, lo:lo + n], raw[:, lo:lo + n],
                                     c2[:, lo:lo + n])
                nc.vector.tensor_mul(swp[:, lo:lo + n], swp[:, lo:lo + n],
                                     s2n[:, lo:lo + n])
                nc.vector.tensor_add(rot[:, lo:lo + n], rot[:, lo:lo + n],
                                     swp[:, lo:lo + n])

            def proj_half(w_h, raw_tag, rot_tag):
                """One projection (Q or K) + RoPE -> rotated [HD, S] tile."""
                raw = w1.tile([P, S], F16, tag=raw_tag)
                for t in range(NQ):
                    ps = pjp.tile([P, 512], F32, tag="pj")
                    for kc in range(KC):
                        nc.tensor.matmul(ps[:], w_h[:, kc, :],
                                         xT[:, t, kc, :],
                                         start=(kc == 0), stop=(kc == KC - 1))
                    nc.scalar.copy(raw[:, ts(t, 512)], ps[:])
                rot = w2.tile([P, S], F16, tag=rot_tag)
                rope_chunk(raw, rot, 0, NQ)
                return rot

            def load_w(h):
                wq_h = w1.tile([P, KC, HD], F16, tag="wqh")
                nc.sync.dma_start(wq_h[:], wq_d[:, h])
                wk_h = w1.tile([P, KC, HD], F16, tag="wkh")
                nc.sync.dma_start(wk_h[:], wk_d[:, h])
                return wq_h, wk_h

            # ---- head-0 projections + V, interleaved ----------------------
            # Per token chunk: K then Q (RoPE chunks run on DVE right after),
            # then the V projections keep the PE busy while DVE rotates and
            # the next xT chunk streams in.
            qraw0 = w1.tile([P, S], F16, tag="qraw")
            kraw0 = w1.tile([P, S], F16, tag="kraw")
            qrot0 = w2.tile([P, S], F16, tag="qrot")
            krot0 = w2.tile([P, S], F16, tag="krot")
            for t in range(NQ):
                for w_h, raw in ((wk_h, kraw0), (wq_h, qraw0)):
                    ps = pjp.tile([P, 512], F32, tag="pj")
                    for kc in range(KC):
                        nc.tensor.matmul(ps[:], w_h[:, kc, :], xT[:, t, kc, :],
                                         start=(kc == 0), stop=(kc == KC - 1))
                    nc.scalar.copy(raw[:, ts(t, 512)], ps[:])
                rope_chunk(kraw0, krot0, t, 1)
                rope_chunk(qraw0, qrot0, t, 1)
                for ti in range(4 * t, 4 * t + 4):
                    ps = pjp.tile([P, 512], F32, tag="pj")
                    for kc in range(KC):
                        nc.tensor.matmul(ps[:], xT[:, t, kc, ts(ti % 4, P)],
                                         wv_sb[:, kc, :],
                                         start=(kc == 0), stop=(kc == KC - 1))
                    nc.scalar.copy(V_sb[:, ti, :], ps[:])
            rots = {0: (qrot0, krot0)}

            flip = [0]

            def emit_f(qi, pool, engines=("vector", "scalar"), split_dma=False):
                """Output projection for token tile qi -> one [P, 2048] DMA.

                engines: PSUM->SBUF copy engines to rotate through.
                (GpSimd cannot access PSUM -- walrus rejects it.)
                """
                f_sb = fop.tile([P, D], F16, tag="fsb")
                for nn in range(D // 512):
                    ps = pool.tile([P, 512], F32, tag="pj")
                    for hh in range(HPC):
                        nc.tensor.matmul(ps[:], OT_sb[:, hh, ts(qi, P)],
                                         wo_sb[:, hh, ts(nn, 512)],
                                         start=(hh == 0), stop=(hh == HPC - 1))
                    eng = engines[flip[0] % len(engines)]
                    if eng == "vector":
                        nc.vector.tensor_copy(f_sb[:, ts(nn, 512)], ps[:])
                    elif eng == "gpsimd":
                        nc.gpsimd.tensor_copy(f_sb[:, ts(nn, 512)], ps[:])
                    else:
                        nc.scalar.copy(f_sb[:, ts(nn, 512)], ps[:])
                    flip[0] += 1
                    # final tiles: drain each half through its own
                    # hardware-DGE queue as soon as its copies are done
                    if split_dma and nn == 1:
                        nc.sync.dma_start(out_d[:, qi, 0:1024], f_sb[:, 0:1024])
                    if split_dma and nn == 3:
                        nc.scalar.dma_start(out_d[:, qi, 1024:2048],
                                            f_sb[:, 1024:2048])
                if not split_dma:
                    nc.sync.dma_start(out_d[:, qi, :], f_sb[:])

            # ---- attention, with next head's projections interleaved ------
            with tc.tile_pool(name="sp", bufs=3, space="PSUM") as stp, \
                 tc.tile_pool(name="op", bufs=2, space="PSUM") as opp, \
                 tc.tile_pool(name="rp", bufs=1, space="PSUM") as rpp:
                pending_fin = [None]
                carry = [[]]
                for h in range(HPC):
                    qrot, krot = rots.pop(h)
                    last = h == HPC - 1
                    # independent PE work units used to plug exp-latency
                    # bubbles in the in-order PE queue: next head's projection
                    # chunks (heads 0..2, during qc==0) or the previous query
                    # chunk's output-projection tiles (last head).
                    if not last:
                        wq_n, wk_n = load_w(h + 1)
                        qraw_n = w1.tile([P, S], F16, tag="qraw")
                        kraw_n = w1.tile([P, S], F16, tag="kraw")
                        qrot_n = w2.tile([P, S], F16, tag="qrot")
                        krot_n = w2.tile([P, S], F16, tag="krot")
                        rots[h + 1] = (qrot_n, krot_n)

                        def proj_unit(w_h, raw, rot, t):
                            def emit():
                                ps = pjp.tile([P, 512], F32, tag="pj")
                                for kc in range(KC):
                                    nc.tensor.matmul(ps[:], w_h[:, kc, :],
                                                     xT[:, t, kc, :],
                                                     start=(kc == 0),
                                                     stop=(kc == KC - 1))
                                nc.scalar.copy(raw[:, ts(t, 512)], ps[:])
                                rope_chunk(raw, rot, t, 1)
                            return emit

                        units = [proj_unit(wk_n, kraw_n, krot_n, t)
                                 for t in range(NQ)]
                        units += [proj_unit(wq_n, qraw_n, qrot_n, t)
                                  for t in range(NQ)]
                        if h == HPC - 2:
                            # hold back two of the last head's projection
                            # chunks: they are its only filler work for the
                            # serial band chain of its first query chunk
                            carry[0] = units[6:]
                            units = units[:6]
                    else:
                        units = carry[0]
                    for qc in range(NQ):
                        o_ps = opp.tile([P, 512], F32, tag="o")
                        r_ps = None
                        if mask_mode != "causal":
                            r_ps = rpp.tile([P, 512], F32, tag="r")
                        nkt = 4 * (qc + 1) if mask_mode == "causal" else NT
                        nfull = 4 * qc if mask_mode == "causal" else 0
                        if last and mask_mode == "causal" and qc > 0:
                            units = [(lambda qi: lambda: emit_f(qi, pjp))(qi)
                                     for qi in range(4 * (qc - 1), 4 * qc)]
                        # full-width tiles accumulate on the DVE into `acc`;
                        # one ones-matmul on the sum replaces one per tile.
                        state = {"acc": None, "first_e": None}
                        pend = {}

                        def emit_score(kt):
                            band = mask_mode == "causal" and kt >= nfull
                            off = 128 * (kt - nfull) if band else 0
                            s_ps = stp.tile([P, 512], F32, tag="s")
                            nc.tensor.matmul(
                                s_ps[:, off:], krot[:, ts(kt, P)],
                                qrot[:, 512 * qc + off: 512 * (qc + 1)],
                                start=True, stop=True)
                            eT = etp.tile([P, 512], F16, tag="e")
                            # bias -4 (softmax is shift-invariant; the ones-
                            # matmul denominator absorbs it) keeps exp outputs
                            # well inside fp16 range even for hot scores
                            nc.scalar.activation(eT[:, off:], s_ps[:, off:], EXP,
                                                 scale=SCALE, bias=bias4[:])
                            if band:
                                # zero the above-diagonal half of the diagonal
                                # 128x128 block (cheap on DVE; GpSimd takes
                                # ~5x longer per op and SWDGE-stalls its queue)
                                nc.vector.tensor_mul(eT[:, off:off + 128],
                                                     eT[:, off:off + 128],
                                                     tri_sb[:])
                            if mask_mode == "general":
                                em = etp.tile([P, 512], F16, tag="em")
                                nc.sync.dma_start(em[:], msk_d[:, kt, ts(qc, 512)])
                                nc.gpsimd.tensor_mul(eT[:], eT[:], em[:])
                            acc, first_e = state["acc"], state["first_e"]
                            if mask_mode != "causal":
                                pass
                            elif not band:
                                if first_e is not None:
                                    acc = accp.tile([P, 512], F16, tag="acc")
                                    nc.vector.tensor_add(acc[:], first_e[:], eT[:])
                                    state["acc"], state["first_e"] = acc, None
                                elif acc is not None:
                                    nc.vector.tensor_add(acc[:], acc[:], eT[:])
                                else:
                                    state["first_e"] = eT
                            elif acc is None and first_e is None:
                                acc = accp.tile([P, 512], F16, tag="acc")
                                nc.vector.tensor_copy(acc[:], eT[:])
                                state["acc"] = acc
                            elif first_e is not None:
                                acc = accp.tile([P, 512], F16, tag="acc")
                                nc.vector.tensor_add(acc[:], first_e[:], eT[:])
                                state["acc"], state["first_e"] = acc, None
                            else:
                                nc.vector.tensor_add(acc[:, off:], acc[:, off:],
                                                     eT[:, off:])
                            pend[kt] = (eT, off)

                        # software-pipeline by one tile: the PE sees
                        # [s0, s1, PV0, s2, PV1, ...] so PV(kt) never waits on
                        # exp(kt) -- the next score matmul runs in the gap.
                        # (Depth 2 regresses badly: a third in-flight score
                        # tile exhausts the sp ring and the allocation wait
                        # serializes the whole PE queue.)
                        emit_score(0)
                        for kt in range(nkt):
                            if kt + 1 < nkt:
                                emit_score(kt + 1)
                            # the previous chunk's denominator chain (ones
                            # matmul -> reciprocal -> OT multiply) runs here,
                            # inside this chunk's score stream, so the PE
                            # never serializes behind the DVE acc chain at a
                            # chunk boundary
                            if kt == 1 and pending_fin[0] is not None:
                                pending_fin[0]()
                                pending_fin[0] = None
                            eT, off = pend.pop(kt)
                            nc.tensor.matmul(o_ps[:, off:],
                                             V_sb[:, kt, ts(h, HD)], eT[:, off:],
                                             start=(kt == 0), stop=(kt == nkt - 1))
                            if mask_mode != "causal":
                                nc.tensor.matmul(r_ps[:], ones_sb[:], eT[:],
                                                 start=(kt == 0),
                                                 stop=(kt == nkt - 1))
                            # last-head emit units read OT written at the END
                            # of the previous chunk (after its DVE rinv+mul
                            # chain): schedule them from kt=4 so the first one
                            # never stalls the PE on that chain
                            ready = (((kt % 3 == 2) if qc == 0 else
                                      (kt >= 5 and (kt - 5) % 3 == 0))
                                     if last else (kt % 3 == 2))
                            if units and ready:
                                units.pop(0)()
                        # the last head's output tiles must flush before the
                        # next query chunk overwrites fq context; projection
                        # units may keep spreading across later query chunks
                        if last:
                            for u in units:
                                u()
                            units = []
                        if mask_mode == "causal":
                            def finalize(h=h, qc=qc, o_ps=o_ps,
                                         acc=state["acc"]):
                                r_ps = rpp.tile([P, 512], F32, tag="r")
                                nc.tensor.matmul(r_ps[:], ones_sb[:], acc[:],
                                                 start=True, stop=True)
                                rinv = rip.tile([P, 512], F32, tag="rinv")
                                nc.vector.reciprocal_approx_fast(out=rinv[:],
                                                                 in_=r_ps[:])
                                nc.vector.tensor_mul(OT_sb[:, h, ts(qc, 512)],
                                                     o_ps[:], rinv[:])
                            if last and qc == NQ - 1:
                                finalize()
                            else:
                                pending_fin[0] = finalize
                        else:
                            rinv = rip.tile([P, 512], F32, tag="rinv")
                            nc.vector.reciprocal_approx_fast(out=rinv[:],
                                                             in_=r_ps[:])
                            nc.vector.tensor_mul(OT_sb[:, h, ts(qc, 512)],
                                                 o_ps[:], rinv[:])
                    # any projection units not consumed by the kt loops
                    for u in units:
                        u()
                    if last and mask_mode == "causal":
                        for qi in range(4 * (NQ - 1), NT):
                            emit_f(qi, pjp, split_dma=True)

            # ---- output projection for non-causal modes (causal streams it
            # inside the last head's attention) --------------------------------
            if mask_mode != "causal":
                with tc.tile_pool(name="fp", bufs=6, space="PSUM") as fpp:
                    for qi in range(NT):
                        emit_f(qi, fpp, engines=("scalar", "vector"))

    nc.compile()
    return nc


def _get_program(mask_mode: str):
    if mask_mode not in _cache:
        _cache[mask_mode] = _build(mask_mode)
    return _cache[mask_mode]


def _detect_mask_mode(mask: np.ndarray) -> str:
    m = mask.reshape(S, S)
    iu = np.triu_indices(S, 1)
    upper = m[iu]
    lower_ok = np.max(np.abs(np.tril(m))) == 0.0
    if lower_ok and upper.size and np.all(upper <= -1e8):
        return "causal"
    if np.max(np.abs(m)) == 0.0:
        return "none"
    return "general"


def _prep_inputs(x, wq, wk, wv, wo, freqs_cos, freqs_sin, mask, mask_mode):
    """Build the 8 per-core input maps (host-side sharding + layout)."""
    # within-head even/odd permutation so RoPE pairs land in partition halves
    perm = np.concatenate([np.arange(0, HD, 2), np.arange(1, HD, 2)])

    cosT = freqs_cos.T.astype(np.float32)          # [64, S]
    sinT = freqs_sin.T.astype(np.float32)
    c2 = np.concatenate([cosT, cosT], 0).astype(NPF16)     # [128, S]
    s2n = np.concatenate([-sinT, sinT], 0).astype(NPF16)
    ones = np.ones((P, P), NPF16)

    common = {"c2": c2, "s2n": s2n, "ones": ones}
    if mask_mode == "causal":
        pp, ff = np.meshgrid(np.arange(P), np.arange(P), indexing="ij")
        common["tri"] = (pp <= ff).astype(NPF16)
    elif mask_mode == "general":
        m = mask.reshape(S, S).astype(np.float32)
        # eT[kt_tok, qt_tok] is multiplied by exp(SCALE * mask[qt_tok, kt_tok])
        expm = np.exp(SCALE * m.T).astype(NPF16)            # [k_tok, q_tok]
        common["expm"] = np.ascontiguousarray(
            expm.reshape(NT, P, S).transpose(1, 0, 2))

    xT_by_b = []
    for b in range(B):
        xT = np.ascontiguousarray(
            x[b].T.reshape(KC, P, NQ, 512).transpose(1, 2, 0, 3)).astype(NPF16)
        xT_by_b.append(xT)

    in_maps = []
    for c in range(NCORES):
        b, g = divmod(c, TP)
        heads = range(g * HPC, (g + 1) * HPC)
        cols_qk = np.concatenate([h * HD + perm for h in heads])
        cols_v = np.concatenate([np.arange(h * HD, (h + 1) * HD) for h in heads])

        wq_c = wq[:, cols_qk].reshape(KC, P, HPC, HD).transpose(1, 2, 0, 3)
        wk_c = wk[:, cols_qk].reshape(KC, P, HPC, HD).transpose(1, 2, 0, 3)
        wv_c = wv[:, cols_v].reshape(KC, P, DVC).transpose(1, 0, 2)
        wo_c = wo[cols_v, :].reshape(HPC, P, D).transpose(1, 0, 2)

        im = dict(common)
        im["xT"] = xT_by_b[b]
        im["wq"] = np.ascontiguousarray(wq_c).astype(NPF16)
        im["wk"] = np.ascontiguousarray(wk_c).astype(NPF16)
        im["wv"] = np.ascontiguousarray(wv_c).astype(NPF16)
        im["wo"] = np.ascontiguousarray(wo_c).astype(NPF16)
        in_maps.append(im)
    return in_maps


def run(inputs: dict, **spmd_kwargs):
    """Run on hardware; returns (output [B,S,D] fp32, BassKernelResults)."""
    x = np.asarray(inputs["x"], np.float32)
    wq = np.asarray(inputs["wq"], np.float32)
    wk = np.asarray(inputs["wk"], np.float32)
    wv = np.asarray(inputs["wv"], np.float32)
    wo = np.asarray(inputs["wo"], np.float32)
    fc = np.asarray(inputs["freqs_cos"], np.float32)
    fs = np.asarray(inputs["freqs_sin"], np.float32)
    mask = np.asarray(inputs["mask"], np.float32)

    mask_mode = _detect_mask_mode(mask)
    nc = _get_program(mask_mode)
    in_maps = _prep_inputs(x, wq, wk, wv, wo, fc, fs, mask, mask_mode)
    res = run_bass_kernel_spmd(nc, in_maps, core_ids=list(range(NCORES)),
                               **spmd_kwargs)

    out = np.zeros((B, S, D), np.float32)
    for c in range(NCORES):
        b = c // TP
        part = res.results[c]["out"].astype(np.float32)   # [P, NT, D]
        out[b] += part.transpose(1, 0, 2).reshape(S, D)
    return out, res


def kernel(**inputs) -> np.ndarray:
    out, _ = run(inputs)
    return out


# revision 42
# speedup vs baseline: 1.0088x; 1.0069x over previous
"""Trainium2 Bass kernel for nn_Attention_62620623176132.

Multi-head causal attention with RoPE (LLaMA-style), B=2, S=2048, D=2048,
H=16 heads of HD=128, fp32 reference.

Sharding (hardcoded): 8 cores = 2-way data parallel over batch x 4-way
tensor parallel over heads (4 heads per core). Each core computes its 4
heads' Q/K/V projections, attention, and a partial output projection
(rows of wo for its heads); the host sums the 4 fp16 partials per batch
in fp32.

Device algorithm (per core; matmuls in fp16 with fp32 PSUM accumulation):
  - x^T kept SBUF-resident; Q^T/K^T computed per head in [HD, S] layout,
    V in [S, dv] layout, so no transposes are ever needed.
  - RoPE via host-side even/odd column permutation of wq/wk: rotation
    pairs land in partition halves; 3 DVE tensor ops + 2 swap copies.
    In the startup loop RoPE runs per 512-token chunk, interleaved with
    the V projections, so the DVE chains hide under PE work.
  - Scores computed transposed, sT[kt, qt] = kT . qT, so exp(sT) feeds
    the PV matmul directly as the moving operand. exp is shifted by -4
    (softmax is shift-invariant) to keep fp16 outputs far from overflow.
  - Softmax denominators: exp tiles are summed on the Vector engine and
    one all-ones stationary matmul per query chunk broadcasts the column
    sums to all partitions; the normalization multiply is fused into the
    PSUM->SBUF copy of the attention output.
  - Causality: score tiles above the diagonal are skipped; band tiles
    are restricted to their unmasked columns and the diagonal square of
    exp values is multiplied by a 0/1 triangle on the Vector engine, so
    masking costs no PE work.
  - Projections for head h+1 are emitted inside head h's attention so
    the serial RoPE chain never stalls the PE; the output projection is
    streamed inside the last head's attention the same way.
  - Output tiles are staged per 128-token tile into a [P, 2048] tile
    and written with a single 4KB-row DMA (4x fewer ~0.6us DMA issues).
  - Startup DMAs are split per 4-kc group and spread across the Sync and
    Scalar hardware-DGE queues in consumption order, so the first K-proj
    matmuls start as soon as the first x^T group lands. (GpSimd DMA is
    the software-DGE path: its transfers start late and stall the Pool
    queue -- never put startup or output DMAs there.)

Fallback paths keyed off the runtime mask: all-zero mask -> non-causal
kernel; any other mask -> multiplicative exp(mask/sqrt(HD)) tiles
streamed from DRAM (correct for arbitrary masks, slower).
"""

import math

import numpy as np
import concourse.tile as tile
import concourse.mybir as mybir
from concourse import bacc
from concourse.bass import ts
from concourse.bass_utils import run_bass_kernel_spmd

B, S, D, H, HD = 2, 2048, 2048, 16, 128
P = 128
NCORES = 8
TP = 4                # head-parallel groups
HPC = H // TP         # heads per core = 4
DVC = HPC * HD        # 512 v-dims per core
KC = D // P           # 16 contraction chunks
NT = S // P           # 16 token tiles of 128
NQ = S // 512         # 4 query chunks of 512
F16 = mybir.dt.float16
F32 = mybir.dt.float32
NPF16 = np.float16
SCALE = 1.0 / math.sqrt(HD)
EXP = mybir.ActivationFunctionType.Exp

_cache: dict = {}


def _build(mask_mode: str):
    """Build + compile the SPMD program. mask_mode: 'causal'|'none'|'general'."""
    nc = bacc.Bacc("TRN2", target_bir_lowering=False, debug=False,
                   num_devices=NCORES)

    def din(name, shape, dt=F16):
        return nc.dram_tensor(name, shape, dt, kind="ExternalInput").ap()

    xT_d = din("xT", [P, NQ, KC, 512])
    wq_d = din("wq", [P, HPC, KC, HD])
    wk_d = din("wk", [P, HPC, KC, HD])
    wv_d = din("wv", [P, KC, DVC])
    wo_d = din("wo", [P, HPC, D])
    c2_d = din("c2", [P, S])
    s2n_d = din("s2n", [P, S])
    ones_d = din("ones", [P, P])
    if mask_mode == "causal":
        tri_d = din("tri", [P, P])
    elif mask_mode == "general":
        msk_d = din("expm", [P, NT, S])
    out_d = nc.dram_tensor("out", [P, NT, D], mybir.dt.float16,
                           kind="ExternalOutput").ap()

    with tile.TileContext(nc) as tc:
        with tc.tile_pool(name="static", bufs=1) as st, \
             tc.tile_pool(name="w1", bufs=1) as w1, \
             tc.tile_pool(name="w2", bufs=2) as w2, \
             tc.tile_pool(name="et", bufs=6) as etp, \
             tc.tile_pool(name="ac", bufs=3) as accp, \
             tc.tile_pool(name="fo", bufs=3) as fop, \
             tc.tile_pool(name="ri", bufs=2) as rip, \
             tc.tile_pool(name="pj", bufs=2, space="PSUM") as pjp:

            # ---- static tensors -------------------------------------------
            xT = st.tile([P, NQ, KC, 512], F16, tag="xT")
            wv_sb = st.tile([P, KC, DVC], F16, tag="wv")
            wo_sb = st.tile([P, HPC, D], F16, tag="wo")
            c2 = st.tile([P, S], F16, tag="c2")
            s2n = st.tile([P, S], F16, tag="s2n")
            ones_sb = st.tile([P, P], F16, tag="ones")
            V_sb = st.tile([P, NT, DVC], F16, tag="V")
            OT_sb = st.tile([P, HPC, S], F16, tag="OT")
            if mask_mode == "causal":
                tri_sb = st.tile([P, P], F16, tag="tri")
            bias4 = st.tile([P, 1], F32, tag="b4")
            nc.vector.memset(bias4[:], -4.0)

            # Startup DMAs: head-0 wk/wq and xT chunk 0 split per 4-kc group
            # so the first projection matmuls start after ~640KB; issues are
            # spread across four engine queues (each costs ~0.6us of issue
            # time on its queue).
            wq_h = w1.tile([P, KC, HD], F16, tag="wqh")
            wk_h = w1.tile([P, KC, HD], F16, tag="wkh")
            # Scalar gets ONLY the 4 small wq chunks: a ring-full DMA wait on
            # the Scalar queue would block the projection PSUM->SBUF copies
            # behind it and starve the PE of PSUM banks. Everything else goes
            # on Sync in strict arrival-priority order (ring-full waits there
            # only delay later, less urgent transfers).
            nc.sync.dma_start(wk_h[:, ts(0, 4), :], wk_d[:, 0, ts(0, 4), :])
            nc.sync.dma_start(xT[:, 0, 0:2, :], xT_d[:, 0, 0:2, :])
            nc.scalar.dma_start(wq_h[:, ts(0, 4), :], wq_d[:, 0, ts(0, 4), :])
            nc.sync.dma_start(xT[:, 0, 2:4, :], xT_d[:, 0, 2:4, :])
            for g in range(1, 4):
                nc.sync.dma_start(wk_h[:, ts(g, 4), :], wk_d[:, 0, ts(g, 4), :])
                nc.sync.dma_start(xT[:, 0, 4 * g:4 * g + 2, :],
                                  xT_d[:, 0, 4 * g:4 * g + 2, :])
                nc.sync.dma_start(xT[:, 0, 4 * g + 2:4 * g + 4, :],
                                  xT_d[:, 0, 4 * g + 2:4 * g + 4, :])
                nc.scalar.dma_start(wq_h[:, ts(g, 4), :], wq_d[:, 0, ts(g, 4), :])
            for g in range(4):
                nc.sync.dma_start(wv_sb[:, ts(g, 4), :], wv_d[:, ts(g, 4), :])
            nc.sync.dma_start(xT[:, 1, :, :], xT_d[:, 1, :, :])
            # c2/s2n feed the DVE rope chains, which have ~50us of slack --
            # xT1 gates PE work at ~28us, so it goes first
            nc.sync.dma_start(c2[:], c2_d)
            nc.sync.dma_start(s2n[:], s2n_d)
            nc.sync.dma_start(xT[:, 2, :, :], xT_d[:, 2, :, :])
            nc.sync.dma_start(xT[:, 3, :, :], xT_d[:, 3, :, :])
            nc.sync.dma_start(ones_sb[:], ones_d)
            if mask_mode == "causal":
                nc.sync.dma_start(tri_sb[:], tri_d)
            nc.sync.dma_start(wo_sb[:], wo_d)

            # PE warm-up: garbage matmuls on a zeroed scratch tile (output
            # never read) fill the ~4us DMA-start latency window so the PE
            # p-state is fully ramped when the first real operands land.
            # (5 matmuls / DVE memset measured best: longer warm-up chains
            # delay the first real matmuls in the in-order PE queue by more
            # than the p-state ramp they save)
            scr = st.tile([P, 512], F16, tag="scr")
            nc.vector.memset(scr[:], 0.0)
            warm_ps = pjp.tile([P, 512], F32, tag="pj")
            for _ in range(5):
                nc.tensor.matmul(warm_ps[:], scr[:, 0:P], scr[:],
                                 start=True, stop=True)

            def rope_chunk(raw, rot, t0, ntc):
                """RoPE on token chunks [t0*512, (t0+ntc)*512) of raw -> rot."""
                lo, n = 512 * t0, 512 * ntc
                swp = w1.tile([P, S], F16, tag="swap")
                nc.vector.tensor_copy(swp[0:64, lo:lo + n], raw[64:128, lo:lo + n])
                nc.vector.tensor_copy(swp[64:128, lo:lo + n], raw[0:64, lo:lo + n])
                nc.vector.tensor_mul(rot[:, lo:lo + n], raw[:, lo:lo + n],
                                     c2[:, lo:lo + n])
                nc.vector.tensor_mul(swp[:, lo:lo + n], swp[:, lo:lo + n],
                                     s2n[:, lo:lo + n])
                nc.vector.tensor_add(rot[:, lo:lo + n], rot[:, lo:lo + n],
                                     swp[:, lo:lo + n])

            def proj_half(w_h, raw_tag, rot_tag):
                """One projection (Q or K) + RoPE -> rotated [HD, S] tile."""
                raw = w1.tile([P, S], F16, tag=raw_tag)
                for t in range(NQ):
                    ps = pjp.tile([P, 512], F32, tag="pj")
                    for kc in range(KC):
                        nc.tensor.matmul(ps[:], w_h[:, kc, :],
                                         xT[:, t, kc, :],
                                         start=(kc == 0), stop=(kc == KC - 1))
                    nc.scalar.copy(raw[:, ts(t, 512)], ps[:])
                rot = w2.tile([P, S], F16, tag=rot_tag)
                rope_chunk(raw, rot, 0, NQ)
                return rot

            def load_w(h):
                wq_h = w1.tile([P, KC, HD], F16, tag="wqh")
                nc.sync.dma_start(wq_h[:], wq_d[:, h])
                wk_h = w1.tile([P, KC, HD], F16, tag="wkh")
                nc.sync.dma_start(wk_h[:], wk_d[:, h])
                return wq_h, wk_h

            # ---- head-0 projections + V, interleaved ----------------------
            # Per token chunk: K then Q (RoPE chunks run on DVE right after),
            # then the V projections keep the PE busy while DVE rotates and
            # the next xT chunk streams in.
            qraw0 = w1.tile([P, S], F16, tag="qraw")
            kraw0 = w1.tile([P, S], F16, tag="kraw")
            qrot0 = w2.tile([P, S], F16, tag="qrot")
            krot0 = w2.tile([P, S], F16, tag="krot")
            for t in range(NQ):
                for w_h, raw in ((wk_h, kraw0), (wq_h, qraw0)):
                    ps = pjp.tile([P, 512], F32, tag="pj")
                    for kc in range(KC):
                        nc.tensor.matmul(ps[:], w_h[:, kc, :], xT[:, t, kc, :],
                                         start=(kc == 0), stop=(kc == KC - 1))
                    nc.scalar.copy(raw[:, ts(t, 512)], ps[:])
                rope_chunk(kraw0, krot0, t, 1)
                rope_chunk(qraw0, qrot0, t, 1)
                for ti in range(4 * t, 4 * t + 4):
                    ps = pjp.tile([P, 512], F32, tag="pj")
                    for kc in range(KC):
                        nc.tensor.matmul(ps[:], xT[:, t, kc, ts(ti % 4, P)],
                                         wv_sb[:, kc, :],
                                         start=(kc == 0), stop=(kc == KC - 1))
                    nc.scalar.copy(V_sb[:, ti, :], ps[:])
            rots = {0: (qrot0, krot0)}

            flip = [0]

            def emit_f(qi, pool, engines=("vector", "scalar"), split_dma=False):
                """Output projection for token tile qi -> one [P, 2048] DMA.

                engines: PSUM->SBUF copy engines to rotate through.
                (GpSimd cannot access PSUM -- walrus rejects it.)
                """
                f_sb = fop.tile([P, D], F16, tag="fsb")
                for nn in range(D // 512):
                    ps = pool.tile([P, 512], F32, tag="pj")
                    for hh in range(HPC):
                        nc.tensor.matmul(ps[:], OT_sb[:, hh, ts(qi, P)],
                                         wo_sb[:, hh, ts(nn, 512)],
                                         start=(hh == 0), stop=(hh == HPC - 1))
                    eng = engines[flip[0] % len(engines)]
                    if eng == "vector":
                        nc.vector.tensor_copy(f_sb[:, ts(nn, 512)], ps[:])
                    elif eng == "gpsimd":
                        nc.gpsimd.tensor_copy(f_sb[:, ts(nn, 512)], ps[:])
                    else:
                        nc.scalar.copy(f_sb[:, ts(nn, 512)], ps[:])
                    flip[0] += 1
                    # final tiles: drain each half through its own
                    # hardware-DGE queue as soon as its copies are done
                    if split_dma and nn == 1:
                        nc.sync.dma_start(out_d[:, qi, 0:1024], f_sb[:, 0:1024])
                    if split_dma and nn == 3:
                        nc.scalar.dma_start(out_d[:, qi, 1024:2048],
                                            f_sb[:, 1024:2048])
                if not split_dma:
                    nc.sync.dma_start(out_d[:, qi, :], f_sb[:])

            # ---- attention, with next head's projections interleaved ------
            with tc.tile_pool(name="sp", bufs=3, space="PSUM") as stp, \
                 tc.tile_pool(name="op", bufs=2, space="PSUM") as opp, \
                 tc.tile_pool(name="rp", bufs=1, space="PSUM") as rpp:
                pending_fin = [None]
                carry = [[]]
                for h in range(HPC):
                    qrot, krot = rots.pop(h)
                    last = h == HPC - 1
                    # independent PE work units used to plug exp-latency
                    # bubbles in the in-order PE queue: next head's projection
                    # chunks (heads 0..2, during qc==0) or the previous query
                    # chunk's output-projection tiles (last head).
                    if not last:
                        wq_n, wk_n = load_w(h + 1)
                        qraw_n = w1.tile([P, S], F16, tag="qraw")
                        kraw_n = w1.tile([P, S], F16, tag="kraw")
                        qrot_n = w2.tile([P, S], F16, tag="qrot")
                        krot_n = w2.tile([P, S], F16, tag="krot")
                        rots[h + 1] = (qrot_n, krot_n)

                        def proj_unit(w_h, raw, rot, t):
                            def emit():
                                ps = pjp.tile([P, 512], F32, tag="pj")
                                for kc in range(KC):
                                    nc.tensor.matmul(ps[:], w_h[:, kc, :],
                                                     xT[:, t, kc, :],
                                                     start=(kc == 0),
                                                     stop=(kc == KC - 1))
                                nc.scalar.copy(raw[:, ts(t, 512)], ps[:])
                                rope_chunk(raw, rot, t, 1)
                            return emit

                        units = [proj_unit(wk_n, kraw_n, krot_n, t)
                                 for t in range(NQ)]
                        units += [proj_unit(wq_n, qraw_n, qrot_n, t)
                                  for t in range(NQ)]
                        if h == HPC - 2:
                            # hold back two of the last head's projection
                            # chunks: they are its only filler work for the
                            # serial band chain of its first query chunk
                            carry[0] = units[6:]
                            units = units[:6]
                    else:
                        units = carry[0]
                    for qc in range(NQ):
                        o_ps = opp.tile([P, 512], F32, tag="o")
                        r_ps = None
                        if mask_mode != "causal":
                            r_ps = rpp.tile([P, 512], F32, tag="r")
                        nkt = 4 * (qc + 1) if mask_mode == "causal" else NT
                        nfull = 4 * qc if mask_mode == "causal" else 0
                        if last and mask_mode == "causal" and qc > 0:
                            units = [(lambda qi: lambda: emit_f(qi, pjp))(qi)
                                     for qi in range(4 * (qc - 1), 4 * qc)]
                        # full-width tiles accumulate on the DVE into `acc`;
                        # one ones-matmul on the sum replaces one per tile.
                        state = {"acc": None, "first_e": None}
                        pend = {}

                        def emit_score(kt):
                            band = mask_mode == "causal" and kt >= nfull
                            off = 128 * (kt - nfull) if band else 0
                            s_ps = stp.tile([P, 512], F32, tag="s")
                            nc.tensor.matmul(
                                s_ps[:, off:], krot[:, ts(kt, P)],
                                qrot[:, 512 * qc + off: 512 * (qc + 1)],
                                start=True, stop=True)
                            eT = etp.tile([P, 512], F16, tag="e")
                            # bias -4 (softmax is shift-invariant; the ones-
                            # matmul denominator absorbs it) keeps exp outputs
                            # well inside fp16 range even for hot scores
                            nc.scalar.activation(eT[:, off:], s_ps[:, off:], EXP,
                                                 scale=SCALE, bias=bias4[:])
                            if band:
                                # zero the above-diagonal half of the diagonal
                                # 128x128 block (cheap on DVE; GpSimd takes
                                # ~5x longer per op and SWDGE-stalls its queue)
                                nc.vector.tensor_mul(eT[:, off:off + 128],
                                                     eT[:, off:off + 128],
                                                     tri_sb[:])
                            if mask_mode == "general":
                                em = etp.tile([P, 512], F16, tag="em")
                                nc.sync.dma_start(em[:], msk_d[:, kt, ts(qc, 512)])
                                nc.gpsimd.tensor_mul(eT[:], eT[:], em[:])
                            acc, first_e = state["acc"], state["first_e"]
                            if mask_mode != "causal":
                                pass
                            elif not band:
                                if first_e is not None:
                                    acc = accp.tile([P, 512], F16, tag="acc")
                                    nc.vector.tensor_add(acc[:], first_e[:], eT[:])
                                    state["acc"], state["first_e"] = acc, None
                                elif acc is not None:
                                    nc.vector.tensor_add(acc[:], acc[:], eT[:])
                                else:
                                    state["first_e"] = eT
                            elif acc is None and first_e is None:
                                acc = accp.tile([P, 512], F16, tag="acc")
                                nc.vector.tensor_copy(acc[:], eT[:])
                                state["acc"] = acc
                            elif first_e is not None:
                                acc = accp.tile([P, 512], F16, tag="acc")
                                nc.vector.tensor_add(acc[:], first_e[:], eT[:])
                                state["acc"], state["first_e"] = acc, None
                            else:
                                nc.vector.tensor_add(acc[:, off:], acc[:, off:],
                                                     eT[:, off:])
                            pend[kt] = (eT, off)

                        # software-pipeline by one tile: the PE sees
                        # [s0, s1, PV0, s2, PV1, ...] so PV(kt) never waits on
                        # exp(kt) -- the next score matmul runs in the gap.
                        # (Depth 2 regresses badly: a third in-flight score
                        # tile exhausts the sp ring and the allocation wait
                        # serializes the whole PE queue.)
                        emit_score(0)
                        for kt in range(nkt):
                            if kt + 1 < nkt:
                                emit_score(kt + 1)
                            # the previous chunk's denominator chain (ones
                            # matmul -> reciprocal -> OT multiply) runs here,
                            # inside this chunk's score stream, so the PE
                            # never serializes behind the DVE acc chain at a
                            # chunk boundary
                            if kt == 1 and pending_fin[0] is not None:
                                pending_fin[0]()
                                pending_fin[0] = None
                            eT, off = pend.pop(kt)
                            nc.tensor.matmul(o_ps[:, off:],
                                             V_sb[:, kt, ts(h, HD)], eT[:, off:],
                                             start=(kt == 0), stop=(kt == nkt - 1))
                            if mask_mode != "causal":
                                nc.tensor.matmul(r_ps[:], ones_sb[:], eT[:],
                                                 start=(kt == 0),
                                                 stop=(kt == nkt - 1))
                            # last-head emit units read OT written at the END
                            # of the previous chunk (after its DVE rinv+mul
                            # chain): schedule them from kt=4 so the first one
                            # never stalls the PE on that chain
                            ready = (((kt % 3 == 2) if qc == 0 else
                                      (kt >= 5 and (kt - 5) % 3 == 0))
                                     if last else (kt % 3 == 2))
                            if units and ready:
                                units.pop(0)()
                        # the last head's output tiles must flush before the
                        # next query chunk overwrites fq context; projection
                        # units may keep spreading across later query chunks
                        if last:
                            for u in units:
                                u()
                            units = []
                        if mask_mode == "causal":
                            def finalize(h=h, qc=qc, o_ps=o_ps,
                                         acc=state["acc"]):
                                r_ps = rpp.tile([P, 512], F32, tag="r")
                                nc.tensor.matmul(r_ps[:], ones_sb[:], acc[:],
                                                 start=True, stop=True)
                                rinv = rip.tile([P, 512], F32, tag="rinv")
                                nc.vector.reciprocal_approx_fast(out=rinv[:],
                                                                 in_=r_ps[:])
                                nc.vector.tensor_mul(OT_sb[:, h, ts(qc, 512)],
                                                     o_ps[:], rinv[:])
                            if last and qc == NQ - 1:
                                finalize()
                            else:
                                pending_fin[0] = finalize
                        else:
                            rinv = rip.tile([P, 512], F32, tag="rinv")
                            nc.vector.reciprocal_approx_fast(out=rinv[:],
                                                             in_=r_ps[:])
                            nc.vector.tensor_mul(OT_sb[:, h, ts(qc, 512)],
                                                 o_ps[:], rinv[:])
                    # any projection units not consumed by the kt loops
                    for u in units:
                        u()
                    if last and mask_mode == "causal":
                        for qi in range(4 * (NQ - 1), NT):
                            emit_f(qi, pjp, split_dma=True)

            # ---- output projection for non-causal modes (causal streams it
            # inside the last head's attention) --------------------------------
            if mask_mode != "causal":
                with tc.tile_pool(name="fp", bufs=6, space="PSUM") as fpp:
                    for qi in range(NT):
                        emit_f(qi, fpp, engines=("scalar", "vector"))

    nc.compile()
    return nc


def _get_program(mask_mode: str):
    if mask_mode not in _cache:
        _cache[mask_mode] = _build(mask_mode)
    return _cache[mask_mode]


def _detect_mask_mode(mask: np.ndarray) -> str:
    m = mask.reshape(S, S)
    iu = np.triu_indices(S, 1)
    upper = m[iu]
    lower_ok = np.max(np.abs(np.tril(m))) == 0.0
    if lower_ok and upper.size and np.all(upper <= -1e8):
        return "causal"
    if np.max(np.abs(m)) == 0.0:
        return "none"
    return "general"


def _prep_inputs(x, wq, wk, wv, wo, freqs_cos, freqs_sin, mask, mask_mode):
    """Build the 8 per-core input maps (host-side sharding + layout)."""
    # within-head even/odd permutation so RoPE pairs land in partition halves
    perm = np.concatenate([np.arange(0, HD, 2), np.arange(1, HD, 2)])

    cosT = freqs_cos.T.astype(np.float32)          # [64, S]
    sinT = freqs_sin.T.astype(np.float32)
    c2 = np.concatenate([cosT, cosT], 0).astype(NPF16)     # [128, S]
    s2n = np.concatenate([-sinT, sinT], 0).astype(NPF16)
    ones = np.ones((P, P), NPF16)

    common = {"c2": c2, "s2n": s2n, "ones": ones}
    if mask_mode == "causal":
        pp, ff = np.meshgrid(np.arange(P), np.arange(P), indexing="ij")
        common["tri"] = (pp <= ff).astype(NPF16)
    elif mask_mode == "general":
        m = mask.reshape(S, S).astype(np.float32)
        # eT[kt_tok, qt_tok] is multiplied by exp(SCALE * mask[qt_tok, kt_tok])
        expm = np.exp(SCALE * m.T).astype(NPF16)            # [k_tok, q_tok]
        common["expm"] = np.ascontiguousarray(
            expm.reshape(NT, P, S).transpose(1, 0, 2))

    xT_by_b = []
    for b in range(B):
        xT = np.ascontiguousarray(
            x[b].T.reshape(KC, P, NQ, 512).transpose(1, 2, 0, 3)).astype(NPF16)
        xT_by_b.append(xT)

    in_maps = []
    for c in range(NCORES):
        b, g = divmod(c, TP)
        heads = range(g * HPC, (g + 1) * HPC)
        cols_qk = np.concatenate([h * HD + perm for h in heads])
        cols_v = np.concatenate([np.arange(h * HD, (h + 1) * HD) for h in heads])

        wq_c = wq[:, cols_qk].reshape(KC, P, HPC, HD).transpose(1, 2, 0, 3)
        wk_c = wk[:, cols_qk].reshape(KC, P, HPC, HD).transpose(1, 2, 0, 3)
        wv_c = wv[:, cols_v].reshape(KC, P, DVC).transpose(1, 0, 2)
        wo_c = wo[cols_v, :].reshape(HPC, P, D).transpose(1, 0, 2)

        im = dict(common)
        im["xT"] = xT_by_b[b]
        im["wq"] = np.ascontiguousarray(wq_c).astype(NPF16)
        im["wk"] = np.ascontiguousarray(wk_c).astype(NPF16)
        im["wv"] = np.ascontiguousarray(wv_c).astype(NPF16)
        im["wo"] = np.ascontiguousarray(wo_c).astype(NPF16)
        in_maps.append(im)
    return in_maps


def run(inputs: dict, **spmd_kwargs):
    """Run on hardware; returns (output [B,S,D] fp32, BassKernelResults)."""
    x = np.asarray(inputs["x"], np.float32)
    wq = np.asarray(inputs["wq"], np.float32)
    wk = np.asarray(inputs["wk"], np.float32)
    wv = np.asarray(inputs["wv"], np.float32)
    wo = np.asarray(inputs["wo"], np.float32)
    fc = np.asarray(inputs["freqs_cos"], np.float32)
    fs = np.asarray(inputs["freqs_sin"], np.float32)
    mask = np.asarray(inputs["mask"], np.float32)

    mask_mode = _detect_mask_mode(mask)
    nc = _get_program(mask_mode)
    in_maps = _prep_inputs(x, wq, wk, wv, wo, fc, fs, mask, mask_mode)
    res = run_bass_kernel_spmd(nc, in_maps, core_ids=list(range(NCORES)),
                               **spmd_kwargs)

    out = np.zeros((B, S, D), np.float32)
    for c in range(NCORES):
        b = c // TP
        part = res.results[c]["out"].astype(np.float32)   # [P, NT, D]
        out[b] += part.transpose(1, 0, 2).reshape(S, D)
    return out, res


def kernel(**inputs) -> np.ndarray:
    out, _ = run(inputs)
    return out
